# revision 16
# baseline (speedup 1.0000x reference)
"""Trainium2 Bass kernel for nn_Embedding2Score (segment_reduce).

Strategy (data-parallel over sessions, per sharding hint):
  - 4096 graphs -> 8 cores x 512 graphs (4 blocks of 128 graphs each).
    Each core owns whole contiguous segments (batch is sorted by graph).
  - Nodes are processed in 512-node supertiles (4 x 128-node subtiles).
    Segment broadcast (v_n -> nodes) and segment sum (alpha*x -> s_g) are
    one-hot matmuls on PE; the one-hot blocks are built with DVE is_equal
    against iota constants (node-major S and graph-major S^T variants).
  - x is shipped in two layouts (node-row packed and feature-row
    transposed) so no on-device transposes are needed; all biases are
    folded into ACT bias / DVE tensor_scalar second operands.
  - Final scoring: s_h^T [128d, 512g] per core vs item_weight^T tiles,
    grouped 4 vocab-tiles per DMA so all phase-2 DMAs are ~1 MB.
    Output rows [512, V] per core = row-slice of [4096, 50000] (no
    collectives).
"""

import sys

if "/opt/trn_rl_repo" not in sys.path:
    sys.path.insert(0, "/opt/trn_rl_repo")

import numpy as np

P = 128          # partitions / tile edge
D = 128          # hidden size
NCORES = 8
NBLK = 4         # graph blocks per core, 128 graphs each
BC = NBLK * P    # graphs per core = 512
VT = 500         # vocab tile (fp32 moving operand <= 512)
VG = 4           # vocab tiles per DMA group
ST = 4           # 128-node subtiles per supertile
NPF = 9          # vocab groups prefetched under phase 1 (non-cc)


def build_nc(ntpb, vpad, repeat=1, phase="both", cc=False):
    """Build the per-core Bass program. ntpb = node tiles per graph-block,
    vpad = padded vocab size (multiple of VT*VG). repeat>1 wraps the body
    in a hardware loop (timing probes). phase: 'both' | 'p1' | 'p2'."""
    import contextlib
    import concourse.bacc as bacc
    import concourse.mybir as mybir
    from concourse.tile import TileContext

    f32 = mybir.dt.float32
    npb = ntpb * P
    nc = bacc.Bacc(num_devices=NCORES) if cc else bacc.Bacc()

    xpk_ext = nc.declare_dram_parameter("xpk", [P, NBLK * npb], f32, isOutput=False)
    xtp_ext = nc.declare_dram_parameter("xtp", [P, NBLK * npb], f32, isOutput=False)
    blc_ext = nc.declare_dram_parameter("blc", [NBLK, P, ntpb], f32, isOutput=False)
    blr_ext = nc.declare_dram_parameter("blr", [NBLK, npb], f32, isOutput=False)
    vnt_ext = nc.declare_dram_parameter("vnt", [D, BC], f32, isOutput=False)
    w1t_ext = nc.declare_dram_parameter("w1t", [D, D], f32, isOutput=False)
    w2t_ext = nc.declare_dram_parameter("w2t", [D, D], f32, isOutput=False)
    w3at_ext = nc.declare_dram_parameter("w3at", [D, D], f32, isOutput=False)
    w3bt_ext = nc.declare_dram_parameter("w3bt", [D, D], f32, isOutput=False)
    b12c_ext = nc.declare_dram_parameter("b12c", [P, 1], f32, isOutput=False)
    w3bc_ext = nc.declare_dram_parameter("w3bc", [P, 1], f32, isOutput=False)
    qwt_ext = nc.declare_dram_parameter("qwt", [D, 1], f32, isOutput=False)
    qbc_ext = nc.declare_dram_parameter("qbc", [P, 1], f32, isOutput=False)
    if cc:
        VS = vpad // NCORES
        itwt_ext = nc.declare_dram_parameter("itws", [D, VS], f32, isOutput=False)
        y_ext = nc.declare_dram_parameter("y", [NCORES * BC, VS], f32, isOutput=True)
        shT_loc = nc.dram_tensor("shT_loc", [D, BC], f32)
        shT_all = nc.dram_tensor("shT_all", [NCORES, D, BC], f32, addr_space="Shared")
    else:
        itwt_ext = nc.declare_dram_parameter("itwt", [D, vpad], f32, isOutput=False)
        y_ext = nc.declare_dram_parameter("y", [BC, vpad], f32, isOutput=True)
        shT_loc = shT_all = None

    with TileContext(nc) as tc:
        with tc.tile_pool(name="const", bufs=1) as cp:
            iota_i = cp.tile([P, P], mybir.dt.int32, tag="iotai")
            nc.gpsimd.iota(iota_i[:], pattern=[[1, P]], base=0, channel_multiplier=0)
            iota_row = cp.tile([P, P], f32, tag="iotarow")
            nc.vector.tensor_copy(out=iota_row[:], in_=iota_i[:])
            iota_ci = cp.tile([P, 1], mybir.dt.int32, tag="iotaci")
            nc.gpsimd.iota(iota_ci[:], pattern=[[0, 1]], base=0, channel_multiplier=1)
            iota_col = cp.tile([P, 1], f32, tag="iotacol")
            nc.vector.tensor_copy(out=iota_col[:], in_=iota_ci[:])

            def load(name, ext, shape):
                t = cp.tile(shape, f32, tag=name)
                nc.sync.dma_start(out=t[:], in_=ext[:])
                return t

            w1t = load("w1t", w1t_ext, [D, D])
            w2t = load("w2t", w2t_ext, [D, D])
            w3at = load("w3at", w3at_ext, [D, D])
            w3bt = load("w3bt", w3bt_ext, [D, D])
            b12c = load("b12c", b12c_ext, [P, 1])
            w3bc = load("w3bc", w3bc_ext, [P, 1])
            qwt = load("qwt", qwt_ext, [D, 1])
            qbc = load("qbc", qbc_ext, [P, 1])
            vnt = load("vnt", vnt_ext, [D, BC])

            shT = cp.tile([D, BC], f32, tag="shT")  # s_h^T, filled per block
            if phase == "p2":
                nc.vector.memset(shT[:], 0.01)
            if cc:
                itws_sb = cp.tile([D, vpad // NCORES], f32, tag="itws")
                nc.sync.dma_start(out=itws_sb[:], in_=itwt_ext[:])
                shAll = cp.tile([D, NCORES * BC], f32, tag="shAll")
                itw_pre = None
            else:
                itws_sb = shAll = None
                itw_pre = []
                W = VG * VT
                for g in range(min(NPF, vpad // W)):
                    t = cp.tile([D, W], f32, tag=f"itwpre{g}")
                    nc.sync.dma_start(out=t[:], in_=itwt_ext[:, g * W:(g + 1) * W])
                    itw_pre.append(t)

            rep_ctx = tc.For_i(0, repeat, 1) if repeat > 1 else contextlib.nullcontext()
            with rep_ctx:
                _build_body(nc, tc, mybir, ntpb, vpad,
                            xpk_ext, xtp_ext, blc_ext, blr_ext, itwt_ext, y_ext,
                            iota_row, iota_col,
                            w1t, w2t, w3at, w3bt, b12c, w3bc, qwt, qbc, vnt, shT,
                            phase, cc, shT_loc, shT_all, itws_sb, shAll,
                            itw_pre)

    nc.compile()
    return nc


def _build_body(nc, tc, mybir, ntpb, vpad,
                xpk_ext, xtp_ext, blc_ext, blr_ext, itwt_ext, y_ext,
                iota_row, iota_col,
                w1t, w2t, w3at, w3bt, b12c, w3bc, qwt, qbc, vnt, shT,
                phase="both", cc=False, shT_loc=None, shT_all=None,
                itws_sb=None, shAll=None, itw_pre=None):
    from concourse.tile import add_dep_helper
    f32 = mybir.dt.float32
    npb = ntpb * P
    nst = -(-ntpb // ST)          # supertiles per block
    Sig = mybir.ActivationFunctionType.Sigmoid
    EQ = mybir.AluOpType.is_equal

    if phase in ("both", "p1"):
        with tc.tile_pool(name="p1big", bufs=2) as pb, \
             tc.tile_pool(name="p1", bufs=4) as pool, \
             tc.tile_pool(name="blkp", bufs=2) as blkp, \
             tc.tile_pool(name="psPre", bufs=3, space="PSUM") as psPre, \
             tc.tile_pool(name="psBlk", bufs=1, space="PSUM") as psBlk, \
             tc.tile_pool(name="psAl", bufs=2, space="PSUM") as psAl, \
             tc.tile_pool(name="psSg", bufs=1, space="PSUM") as psSg:
            for blk in range(NBLK):
                gsl = slice(blk * P, (blk + 1) * P)
                nsl = slice(blk * npb, (blk + 1) * npb)
                xpk = pb.tile([P, npb], f32, tag="xpk")
                nc.sync.dma_start(out=xpk[:], in_=xpk_ext[:, nsl])
                xtp = pb.tile([P, npb], f32, tag="xtp")
                nc.sync.dma_start(out=xtp[:], in_=xtp_ext[:, nsl])
                blc = blkp.tile([P, ntpb], f32, tag="blc")
                nc.sync.dma_start(out=blc[:], in_=blc_ext[blk])
                # q1g[g, d] = (v_n_blk @ W1_w.T)[g, d]  (biases folded later)
                q1g_ps = psBlk.tile([P, P], f32, tag="blkmm", space="PSUM")
                nc.tensor.matmul(out=q1g_ps[:], lhsT=vnt[:, gsl], rhs=w1t[:],
                                 start=True, stop=True)
                q1g = blkp.tile([P, P], f32, tag="q1g")
                nc.vector.tensor_copy(out=q1g[:], in_=q1g_ps[:])

                sg_ps = psSg.tile([P, P], f32, tag="sg", space="PSUM")
                mm_i = 0
                n_mm = sum(min(ST, ntpb - ST * s) for s in range(nst))
                for st in range(nst):
                    nsub = min(ST, ntpb - ST * st)
                    w = nsub * P
                    ssl = slice(st * ST * P, st * ST * P + w)  # cols in block
                    # batchloc broadcast down partitions (DMA stride-0 read)
                    bcb = pool.tile([P, ST * P], f32, tag="bcb")
                    nc.sync.dma_start(
                        out=bcb[:, :w],
                        in_=blr_ext[blk:blk + 1, ssl].to_broadcast((P, w)))
                    # S^T[g, n] = (batchloc[n] == g)   [one op, 512 wide]
                    StT = pool.tile([P, ST * P], f32, tag="StT")
                    nc.vector.tensor_scalar(out=StT[:, :w], in0=bcb[:, :w],
                                            scalar1=iota_col[:], scalar2=None,
                                            op0=EQ)
                    # S[n, g] per 128-node subtile
                    S_st = pool.tile([P, ST * P], f32, tag="S")
                    for c in range(nsub):
                        csl = slice(c * P, (c + 1) * P)
                        nc.vector.tensor_scalar(
                            out=S_st[:, csl], in0=iota_row[:],
                            scalar1=blc[:, st * ST + c:st * ST + c + 1],
                            scalar2=None, op0=EQ)
                    # pre^T[d, n] = W2 @ x^T + q1g^T-expand   (+b12 in ACT)
                    pre_ps = psPre.tile([P, ST * P], f32, tag="pre", space="PSUM")
                    nc.tensor.matmul(out=pre_ps[:, :w], lhsT=w2t[:],
                                     rhs=xtp[:, ssl], start=True, stop=False)
                    nc.tensor.matmul(out=pre_ps[:, :w], lhsT=q1g[:],
                                     rhs=StT[:, :w], start=False, stop=True)
                    sigT = pool.tile([P, ST * P], f32, tag="sigT")
                    nc.scalar.activation(out=sigT[:, :w], in_=pre_ps[:, :w],
                                         func=Sig, bias=b12c[:])
                    # alpha[n] = sig @ q_w.T (+ q_b in copy)
                    al_ps = psAl.tile([P, ST], f32, tag="al", space="PSUM")
                    for c in range(nsub):
                        csl = slice(c * P, (c + 1) * P)
                        nc.tensor.matmul(out=al_ps[:, c:c + 1],
                                         lhsT=sigT[:, csl], rhs=qwt[:],
                                         start=True, stop=True)
                    al = pool.tile([P, ST], f32, tag="al")
                    nc.vector.tensor_scalar_add(out=al[:, :nsub],
                                                in0=al_ps[:, :nsub],
                                                scalar1=qbc[:])
                    # xa = alpha * x ; s_g^T[d, g] += xa^T-reduce via S
                    xa = pool.tile([P, ST * P], f32, tag="xa")
                    for c in range(nsub):
                        csl = slice(c * P, (c + 1) * P)
                        nc.vector.tensor_scalar_mul(
                            out=xa[:, csl],
                            in0=xpk[:, st * ST * P + c * P:st * ST * P + (c + 1) * P],
                            scalar1=al[:, c:c + 1])
                        nc.tensor.matmul(out=sg_ps[:], lhsT=xa[:, csl],
                                         rhs=S_st[:, csl],
                                         start=(mm_i == 0), stop=(mm_i == n_mm - 1))
                        mm_i += 1

                sg_sb = blkp.tile([P, P], f32, tag="sgsb")
                nc.vector.tensor_copy(out=sg_sb[:], in_=sg_ps[:])
                # s_h^T[d, g] = W3a @ v_n^T + W3b @ s_g^T  (+W3_b in copy)
                sh_ps = psBlk.tile([P, P], f32, tag="blkmm", space="PSUM")
                nc.tensor.matmul(out=sh_ps[:], lhsT=w3at[:], rhs=vnt[:, gsl],
                                 start=True, stop=False)
                nc.tensor.matmul(out=sh_ps[:], lhsT=w3bt[:], rhs=sg_sb[:],
                                 start=False, stop=True)
                nc.vector.tensor_scalar_add(out=shT[:, gsl], in0=sh_ps[:],
                                            scalar1=w3bc[:])

    if cc and phase in ("both", "p2"):
        # all-gather s_h^T across the 8 cores, then score the local vocab
        # shard (item weights are fully SBUF-resident) for ALL 4096 sessions
        dma_loc = nc.sync.dma_start(out=shT_loc[:], in_=shT[:])
        cc_i = nc.gpsimd.collective_compute(
            "AllGather", mybir.AluOpType.bypass,
            replica_groups=[list(range(NCORES))],
            ins=[shT_loc[:]], outs=[shT_all[:]])
        add_dep_helper(cc_i.ins, dma_loc.ins, reason="allgather after shT store")
        for c in range(NCORES):
            d = nc.sync.dma_start(out=shAll[:, c * BC:(c + 1) * BC],
                                  in_=shT_all[c])
            add_dep_helper(d.ins, cc_i.ins, reason="read gathered shT")
        VS = vpad // NCORES
        tw = [VT] * (VS // VT) + ([VS % VT] if VS % VT else [])
        grps = [tw[i:i + VG] for i in range(0, len(tw), VG)]
        with tc.tile_pool(name="p2o", bufs=6) as p2o, \
             tc.tile_pool(name="ps2", bufs=3, space="PSUM") as ps2:
            for sb in range(NCORES * NBLK):
                ssl = slice(sb * P, (sb + 1) * P)
                off = 0
                for grp in grps:
                    gw = sum(grp)
                    sc = p2o.tile([P, VG * VT], f32, tag="scsb")
                    so = 0
                    for w in grp:
                        sc_ps = ps2.tile([P, VT], f32, tag="sc", space="PSUM")
                        nc.tensor.matmul(out=sc_ps[:, :w], lhsT=shAll[:, ssl],
                                         rhs=itws_sb[:, off + so:off + so + w],
                                         start=True, stop=True)
                        if (so // VT) % 2 == 0:
                            nc.vector.tensor_copy(out=sc[:, so:so + w],
                                                  in_=sc_ps[:, :w])
                        else:
                            nc.scalar.copy(out=sc[:, so:so + w], in_=sc_ps[:, :w])
                        so += w
                    nc.sync.dma_start(out=y_ext[ssl, off:off + gw],
                                      in_=sc[:, :gw])
                    off += gw

    if (not cc) and phase in ("both", "p2"):
        with tc.tile_pool(name="p2", bufs=4) as p2, \
             tc.tile_pool(name="p2o", bufs=6) as p2o, \
             tc.tile_pool(name="ps2", bufs=4, space="PSUM") as ps2:
            # ------------- phase 2: scores = s_h @ item_weight^T ----------
            W = VG * VT
            ngrp = vpad // W
            for g in range(ngrp):
                gvsl = slice(g * W, (g + 1) * W)
                if itw_pre is not None and g < len(itw_pre):
                    itw = itw_pre[g]
                else:
                    itw = p2.tile([D, W], f32, tag="itw")
                    nc.sync.dma_start(out=itw[:], in_=itwt_ext[:, gvsl])
                for blk in range(NBLK):
                    gsl = slice(blk * P, (blk + 1) * P)
                    sc = p2o.tile([P, W], f32, tag="scsb")
                    for s in range(VG):
                        sc_ps = ps2.tile([P, VT], f32, tag="sc", space="PSUM")
                        nc.tensor.matmul(out=sc_ps[:],
                                         lhsT=shT[:, gsl],
                                         rhs=itw[:, s * VT:(s + 1) * VT],
                                         start=True, stop=True)
                        if s % 2 == 0:
                            nc.vector.tensor_copy(out=sc[:, s * VT:(s + 1) * VT],
                                                  in_=sc_ps[:])
                        else:
                            nc.scalar.copy(out=sc[:, s * VT:(s + 1) * VT],
                                           in_=sc_ps[:])
                    nc.sync.dma_start(out=y_ext[blk * P:(blk + 1) * P, gvsl],
                                      in_=sc[:])


def prep_inputs(session_embedding, item_weight, W1_w, W1_b, W2_w, W2_b,
                q_w, q_b, W3_w, W3_b, batch, num_graphs):
    """Host-side sharding/layout. Returns (in_maps, ntpb, vpad, V)."""
    x = np.ascontiguousarray(np.asarray(session_embedding, dtype=np.float32))
    itw = np.asarray(item_weight, dtype=np.float32)
    batch = np.asarray(batch).astype(np.int64)
    B = int(num_graphs)
    N, d = x.shape
    V = itw.shape[0]
    assert d == D and B == NCORES * BC, (d, B)

    counts = np.bincount(batch, minlength=B)
    assert counts.min() >= 1, "every graph must be non-empty"
    starts = np.zeros(B + 1, np.int64)
    np.cumsum(counts, out=starts[1:])
    assert starts[-1] == N
    last_idx = starts[1:] - 1
    v_n = x[last_idx]                                   # [B, D]

    blk_cnt = starts[P::P] - starts[:-P:P].reshape(-1)  # [B//P]
    ntpb = int(-(-blk_cnt.max() // P))                  # ceil
    npb = ntpb * P

    # In-NEFF AllGather desyncs the axon PJRT mesh (collectives unsupported
    # on this execution path) -- keep the no-collective row-sharded plan.
    cc = False
    if cc:
        vpad = V
    else:
        vpad = -(-V // (VT * VG)) * (VT * VG)
    itwT = np.zeros((D, vpad), np.float32)
    itwT[:, :V] = itw.T

    w1t = np.ascontiguousarray(np.asarray(W1_w, np.float32).T)
    w2t = np.ascontiguousarray(np.asarray(W2_w, np.float32).T)
    W3 = np.asarray(W3_w, np.float32)
    w3at = np.ascontiguousarray(W3[:, :D].T)
    w3bt = np.ascontiguousarray(W3[:, D:].T)
    b12c = (np.asarray(W1_b, np.float32) + np.asarray(W2_b, np.float32)
            ).reshape(P, 1).copy()
    w3bc = np.asarray(W3_b, np.float32).reshape(P, 1).copy()
    qwt = np.ascontiguousarray(np.asarray(q_w, np.float32).reshape(1, D).T)
    qbc = np.full((P, 1), np.float32(np.asarray(q_b).reshape(())), np.float32)

    in_maps = []
    for c in range(NCORES):
        xpad = np.zeros((NBLK, npb, D), np.float32)
        bl = np.zeros((NBLK, P, ntpb), np.float32)
        blr = np.zeros((NBLK, npb), np.float32)
        for b in range(NBLK):
            glo = c * BC + b * P
            s, e = int(starts[glo]), int(starts[glo + P])
            cnt = e - s
            assert cnt <= npb
            xpad[b, :cnt] = x[s:e]
            locp = np.zeros(npb, np.float32)
            locp[:cnt] = (batch[s:e] - glo).astype(np.float32)
            bl[b] = locp.reshape(ntpb, P).T
            blr[b] = locp
        # packed node-row: xpk[:, blk*npb + t*128 + j][i] = x_pad[blk, t*128+i, j]
        xpk = np.ascontiguousarray(
            xpad.reshape(NBLK, ntpb, P, D).transpose(2, 0, 1, 3).reshape(P, NBLK * npb))
        # feature-row transposed: xtp[:, blk*npb + n] = x_pad[blk, n, :]
        xtp = np.ascontiguousarray(
            xpad.transpose(2, 0, 1).reshape(P, NBLK * npb))
        vnt = np.ascontiguousarray(v_n[c * BC:(c + 1) * BC].T)
        im = dict(
            xpk=xpk, xtp=xtp, blc=np.ascontiguousarray(bl),
            blr=np.ascontiguousarray(blr), vnt=vnt,
            w1t=w1t, w2t=w2t, w3at=w3at, w3bt=w3bt,
            b12c=b12c, w3bc=w3bc, qwt=qwt, qbc=qbc)
        if cc:
            VS = V // NCORES
            im["itws"] = np.ascontiguousarray(itwT[:, c * VS:(c + 1) * VS])
        else:
            im["itwt"] = itwT
        in_maps.append(im)
    return in_maps, ntpb, vpad, V, cc


_NC_CACHE = {}


def get_nc(ntpb, vpad, repeat=1, phase="both", cc=False):
    key = (ntpb, vpad, repeat, phase, cc)
    if key not in _NC_CACHE:
        _NC_CACHE[key] = build_nc(ntpb, vpad, repeat, phase, cc)
    return _NC_CACHE[key]


def kernel(**inputs):
    from concourse.bass_utils import run_bass_kernel_spmd

    in_maps, ntpb, vpad, V, cc = prep_inputs(**inputs)
    nc = get_nc(ntpb, vpad, cc=cc)
    res = run_bass_kernel_spmd(nc, in_maps, core_ids=list(range(NCORES)))
    B = int(inputs["num_graphs"])
    y = np.empty((B, V), np.float32)
    if cc:
        VS = V // NCORES
        for c in range(NCORES):
            y[:, c * VS:(c + 1) * VS] = res.results[c]["y"]
    else:
        for c in range(NCORES):
            y[c * BC:(c + 1) * BC] = res.results[c]["y"][:, :V]
    return y
